# revision 5
# baseline (speedup 1.0000x reference)
"""Trainium2 Bass kernel for nn_BiasedMultiHeadAtten (8-core SPMD, tensor
parallel over heads).

The torch module's transpose(0,1)+reshape "scramble" means head n of the
attention only reads rows [64n,64n+64) u [1024+64n,1024+64n+64) of q/k, and
the per-head attention factors into four 1024x1024 score blocks with
contraction 64.  Sharding 2 heads per core therefore also shards the q/k
projections 8-way (256 of 2048 rows each).

Per core c (heads 2c, 2c+1):
  - project q,k for its 256 rows (contraction 4096, via PE fp32r)
  - scrambled attention: S^T = Y^T X per (a0,b0,b1-block), exp on ACT
    (no max subtraction: |scores| <= ~11), AV via PE with a ones-column
    appended to V^T producing the softmax denominators for free
  - out-proj partial: o_cols @ Wo[:,cols]^T  (full 2048 rows)
  - gated-residual branch for its 256 rows, added into the two row-tiles
    that the host-side feature permutation pins at tile 0/1
Host sums the 8 partial outputs with per-core row un-permutation.
"""

import numpy as np

import concourse.bacc as bacc
import concourse.mybir as mybir
import concourse.tile as tile
from concourse import bass_utils

N_CORES = 8
L, H, E, E2, HD = 2048, 1024, 4096, 2048, 64
F32 = mybir.dt.float32
F32R = mybir.dt.float32r
AF = mybir.ActivationFunctionType
ADD = mybir.AluOpType.add
MULT = mybir.AluOpType.mult

_NC_CACHE = {}


def _perm16(c):
    """Block permutation: device l-tile j holds global l-tile perm[j];
    perm[0] = c and perm[1] = 8 + c so the residual rows sit at tiles 0,1."""
    perm = list(range(16))

    def place(pos, val):
        i = perm.index(val)
        perm[pos], perm[i] = perm[i], perm[pos]

    place(0, c)
    place(1, 8 + c)
    return perm


def _emit(nc, tc, d, out):
    from contextlib import ExitStack

    with ExitStack() as ctx:
        pers = ctx.enter_context(tc.tile_pool(name="pers", bufs=1))

        ident = pers.tile([128, 128], F32R, tag="ident", name="ident")
        nc.sync.dma_start(ident[:], d["ident"][:])
        wo_sb = pers.tile([128, H], F32R, tag="wo", name="wo")
        nc.sync.dma_start(wo_sb[:], d["WoT"][:])
        ones_sb = pers.tile([128, 2], F32R, tag="ones", name="ones")
        nc.sync.dma_start(ones_sb[:], d["ones"][:])
        bias = {}
        for bn in ("bqb", "bkb", "blinb", "bres2b", "bob"):
            bias[bn] = pers.tile([128, H], F32, tag=bn, name=bn)
            nc.sync.dma_start(bias[bn][:], d[bn][:])

        X = [pers.tile([128, H], F32R, tag=f"X{a}", name=f"X{a}") for a in range(2)]
        Y = [pers.tile([128, H], F32R, tag=f"Y{b}", name=f"Y{b}") for b in range(2)]
        VT = [[pers.tile([128, 130], F32R, tag=f"VT{b}_{j}", name=f"VT{b}_{j}") for j in range(8)]
              for b in range(2)]
        ocolsT = pers.tile([128, 1024, 2], F32R, tag="ocolsT", name="ocolsT")

        # ---- Phase A: q/k projections for this core's 256 rows ----------
        with tc.tile_pool(name="phA", bufs=3) as pA, \
             tc.tile_pool(name="phN", bufs=1) as pN, \
             tc.tile_pool(name="psA", bufs=1, space="PSUM") as psA:
            nodeT_sb = []
            for e in range(32):
                t = pN.tile([128, 256], F32R, tag=f"node{e}", name=f"node{e}")
                nc.sync.dma_start(t[:], d["nodeT"][128 * e:128 * (e + 1), :])
                nodeT_sb.append(t)
            qps = [[psA.tile([128, 512], F32, tag=f"q{lb}{ch}", name=f"q{lb}{ch}")
                    for ch in range(2)] for lb in range(2)]
            kps = [[psA.tile([128, 512], F32, tag=f"k{lb}{ch}", name=f"k{lb}{ch}")
                    for ch in range(2)] for lb in range(2)]
            for e in range(32):
                wq = pA.tile([128, H], F32R, tag="wq", name="wq")
                nc.sync.dma_start(wq[:], d["WqT"][128 * e:128 * (e + 1), :])
                wk = pA.tile([128, H], F32R, tag="wk", name="wk")
                nc.sync.dma_start(wk[:], d["WkT"][128 * e:128 * (e + 1), :])
                st, sp = (e == 0), (e == 31)
                for lb in range(2):
                    lhs = nodeT_sb[e][:, 128 * lb:128 * (lb + 1)]
                    for ch in range(2):
                        nc.tensor.matmul(qps[lb][ch][:], lhs,
                                         wq[:, 512 * ch:512 * (ch + 1)],
                                         start=st, stop=sp)
                    for ch in range(2):
                        nc.tensor.matmul(kps[lb][ch][:], lhs,
                                         wk[:, 512 * ch:512 * (ch + 1)],
                                         start=st, stop=sp)
            for lb in range(2):
                for ch in range(2):
                    sl = slice(512 * ch, 512 * (ch + 1))
                    nc.vector.tensor_add(X[lb][:, sl], qps[lb][ch][:],
                                         bias["bqb"][:, sl])
                    nc.vector.tensor_add(Y[lb][:, sl], kps[lb][ch][:],
                                         bias["bkb"][:, sl])

        # ---- Phase B: V^T tiles (PE transpose), ones col for denom ------
        with tc.tile_pool(name="psT", bufs=2, space="PSUM") as psT:
            for b0 in range(2):
                for j in range(8):
                    pt = psT.tile([128, 128], F32R, tag="tp", name="tp")
                    nc.tensor.transpose(pt[:], Y[b0][:, 128 * j:128 * (j + 1)],
                                        ident[:])
                    vt = VT[b0][j]
                    nc.vector.tensor_copy(vt[:, 0:64], pt[:, 0:64])
                    nc.vector.tensor_copy(vt[:, 64:65], ones_sb[:, 0:1])
                    nc.vector.tensor_copy(vt[:, 65:129], pt[:, 64:128])
                    nc.vector.tensor_copy(vt[:, 129:130], ones_sb[:, 1:2])

        # ---- Phase C: scrambled attention ------------------------------
        with tc.tile_pool(name="pP", bufs=2) as pP, \
             tc.tile_pool(name="pM", bufs=2) as pM, \
             tc.tile_pool(name="psS", bufs=1, space="PSUM") as psS, \
             tc.tile_pool(name="psO", bufs=1, space="PSUM") as psO:
            for a0 in range(2):
                O_ps = [[psO.tile([65, 512], F32, tag=f"O{h}{ch}", name=f"O{h}{ch}")
                         for ch in range(2)] for h in range(2)]
                for b0 in range(2):
                    for j in range(8):
                        bt = 8 * b0 + j
                        s_ps = [psS.tile([128, 1024], F32, tag=f"s{h}", name=f"s{h}")
                                for h in range(2)]
                        for h in range(2):
                            hp = slice(64 * h, 64 * (h + 1))
                            for ch in range(2):
                                nc.tensor.matmul(
                                    s_ps[h][:, 512 * ch:512 * (ch + 1)],
                                    Y[b0][hp, 128 * j:128 * (j + 1)],
                                    X[a0][hp, 512 * ch:512 * (ch + 1)],
                                    start=True, stop=True)
                        p_sb = [pP.tile([128, 1024], F32R, tag=f"p{h}", name=f"p{h}")
                                for h in range(2)]
                        for h in range(2):
                            nc.scalar.activation(p_sb[h][:], s_ps[h][:],
                                                 AF.Exp, scale=0.125)
                        for h in range(2):
                            for ch in range(2):
                                nc.tensor.matmul(
                                    O_ps[h][ch][:],
                                    VT[b0][j][:, 65 * h:65 * (h + 1)],
                                    p_sb[h][:, 512 * ch:512 * (ch + 1)],
                                    start=(bt == 0), stop=(bt == 15))
                # normalize by the ones-row denominators, scatter into ocolsT
                for h in range(2):
                    r_sb = pM.tile([1, 1024], F32, tag="r", name="r")
                    for ch in range(2):
                        nc.vector.tensor_copy(r_sb[:, 512 * ch:512 * (ch + 1)],
                                              O_ps[h][ch][64:65, :])
                    rcp = pM.tile([1, 1024], F32, tag="rcp", name="rcp")
                    nc.vector.reciprocal(rcp[:], r_sb[:])
                    rcpb = pM.tile([64, 1024], F32, tag="rcpb", name="rcpb")
                    nc.gpsimd.partition_broadcast(rcpb[:], rcp[:])
                    for ch in range(2):
                        nc.vector.tensor_mul(
                            ocolsT[64 * h:64 * (h + 1),
                                   512 * ch:512 * (ch + 1), a0],
                            O_ps[h][ch][0:64, :],
                            rcpb[:, 512 * ch:512 * (ch + 1)])

        # ---- Phase R: gated residual branch for this core's 256 rows ----
        resg = [pers.tile([128, H], F32, tag=f"resg{lb}", name=f"resg{lb}") for lb in range(2)]
        with tc.tile_pool(name="pR", bufs=2) as pR, \
             tc.tile_pool(name="pAB", bufs=1) as pAB, \
             tc.tile_pool(name="psR", bufs=2, space="PSUM") as psR, \
             tc.tile_pool(name="psR2", bufs=1, space="PSUM") as psR2:
            abt_sb = []
            for t in range(16):
                a = pAB.tile([128, 256], F32R, tag=f"ab{t}", name=f"ab{t}")
                nc.sync.dma_start(a[:], d["abT"][128 * t:128 * (t + 1), :])
                abt_sb.append(a)
            res1T = [pers.tile([128, 256], F32R, tag=f"r1_{hb}", name=f"r1_{hb}")
                     for hb in range(8)]
            # res1^T[h, l'] accumulation, h-block outer / e2 inner
            for hb in range(8):
                wl = pR.tile([128, 16, 128], F32R, tag="wl", name="wl")
                nc.sync.dma_start(wl[:], d["WlinT8"][hb])
                rp = psR.tile([128, 256], F32, tag="rp", name="rp")
                for t in range(16):
                    nc.tensor.matmul(rp[:], wl[:, t, :], abt_sb[t][:],
                                     start=(t == 0), stop=(t == 15))
                nc.vector.tensor_copy(res1T[hb][:], rp[:])
            # res2 = res1' @ Wres^T (+ blin folded into bres2b)
            rp2 = [[psR2.tile([128, 512], F32, tag=f"rp2{lb}{ch}", name=f"rp2{lb}{ch}")
                    for ch in range(2)] for lb in range(2)]
            for hb in range(8):
                wr = pR.tile([128, H], F32R, tag="wr", name="wr")
                nc.sync.dma_start(wr[:], d["WresT"][128 * hb:128 * (hb + 1), :])
                for lb in range(2):
                    for ch in range(2):
                        nc.tensor.matmul(rp2[lb][ch][:],
                                         res1T[hb][:, 128 * lb:128 * (lb + 1)],
                                         wr[:, 512 * ch:512 * (ch + 1)],
                                         start=(hb == 0), stop=(hb == 7))
            g_sb = []
            for lb in range(2):
                tt = pR.tile([128, H], F32, tag="tt", name="tt")
                for ch in range(2):
                    sl = slice(512 * ch, 512 * (ch + 1))
                    nc.vector.tensor_add(tt[:, sl], rp2[lb][ch][:],
                                         bias["bres2b"][:, sl])
                g = pers.tile([128, H], F32, tag=f"g{lb}", name=f"g{lb}")
                nc.scalar.activation(g[:], tt[:], AF.Sigmoid)
                g_sb.append(g)
            # res = (res1 + blin) * sigmoid(res2), transposed to (l', h)
            for lb in range(2):
                for hb in range(8):
                    tp = psR.tile([128, 128], F32R, tag="tp2", name="tp2")
                    nc.tensor.transpose(tp[:],
                                        res1T[hb][:, 128 * lb:128 * (lb + 1)],
                                        ident[:])
                    sl = slice(128 * hb, 128 * (hb + 1))
                    nc.vector.tensor_add(resg[lb][:, sl], tp[:],
                                         bias["blinb"][:, sl])
                    nc.vector.tensor_mul(resg[lb][:, sl], resg[lb][:, sl],
                                         g_sb[lb][:, sl])
                nc.vector.tensor_add(resg[lb][:], resg[lb][:], bias["bob"][:])

        # ---- Phase O: out-projection partial + residual/bias add --------
        with tc.tile_pool(name="pO", bufs=2) as pO, \
             tc.tile_pool(name="psF", bufs=2, space="PSUM") as psF:
            oc_flat = ocolsT[:].rearrange("p a b -> p (a b)")
            for j in range(16):
                op = psF.tile([128, 1024], F32, tag="op", name="op")
                for ch in range(2):
                    nc.tensor.matmul(op[:, 512 * ch:512 * (ch + 1)],
                                     oc_flat[:, 128 * j:128 * (j + 1)],
                                     wo_sb[:, 512 * ch:512 * (ch + 1)],
                                     start=True, stop=True)
                ob = pO.tile([128, H], F32, tag="ob", name="ob")
                if j < 2:
                    nc.vector.tensor_add(ob[:], op[:], resg[j][:])
                else:
                    nc.vector.tensor_copy(ob[:], op[:])
                nc.sync.dma_start(out[128 * j:128 * (j + 1), :], ob[:])


def _build_nc():
    nc = bacc.Bacc("TRN2", target_bir_lowering=False, debug=False,
                   num_devices=N_CORES)
    d = {}

    def din(name, shape, dt=F32R):
        d[name] = nc.dram_tensor(name, shape, dt, kind="ExternalInput").ap()

    din("nodeT", (E, 256))
    din("WqT", (E, H))
    din("WkT", (E, H))
    din("abT", (E2, 256))
    din("WlinT8", (8, 128, 16, 128))
    din("WresT", (H, H))
    din("WoT", (128, H))
    din("ident", (128, 128))
    din("ones", (128, 2))
    for bn in ("bqb", "bkb", "blinb", "bres2b", "bob"):
        din(bn, (128, H), F32)
    out = nc.dram_tensor("out", (L, H), F32, kind="ExternalOutput").ap()
    with tile.TileContext(nc) as tc:
        _emit(nc, tc, d, out)
    nc.compile()
    return nc


def get_nc():
    if "nc" not in _NC_CACHE:
        _NC_CACHE["nc"] = _build_nc()
    return _NC_CACHE["nc"]


def build_in_maps(inputs):
    f32 = np.float32
    ne = np.asarray(inputs["node_embedding"], f32)
    ab = np.asarray(inputs["atten_bias"], f32)
    Wq = np.asarray(inputs["Wq"], f32)
    Wk = np.asarray(inputs["Wk"], f32)
    Wlin = np.asarray(inputs["Wlin"], f32)
    Wres = np.asarray(inputs["Wres"], f32)
    Wo = np.asarray(inputs["Wo"], f32)
    bq = np.asarray(inputs["bq"], f32)
    bk = np.asarray(inputs["bk"], f32)
    blin = np.asarray(inputs["blin"], f32)
    bres = np.asarray(inputs["bres"], f32)
    bo = np.asarray(inputs["bo"], f32)

    WkT = np.ascontiguousarray(Wk.T)
    WlinT8 = np.ascontiguousarray(
        Wlin.T.reshape(16, 128, 8, 128).transpose(2, 1, 0, 3))
    WresT = np.ascontiguousarray(Wres.T)
    ident = np.eye(128, dtype=f32)
    bres2 = Wres @ blin + bres
    bkb = np.ascontiguousarray(np.broadcast_to(bk, (128, H)))
    blinb = np.ascontiguousarray(np.broadcast_to(blin, (128, H)))
    bres2b = np.ascontiguousarray(np.broadcast_to(bres2, (128, H)))
    bob = np.ascontiguousarray(np.broadcast_to(bo, (128, H)))

    in_maps = []
    for c in range(N_CORES):
        rows = np.r_[128 * c:128 * (c + 1),
                     1024 + 128 * c:1024 + 128 * (c + 1)]
        colperm = np.concatenate([np.arange(64) + 64 * p for p in _perm16(c)])
        in_maps.append({
            "nodeT": np.ascontiguousarray(ne[rows].T),
            "WqT": np.ascontiguousarray(Wq.T[:, colperm]),
            "WkT": WkT,
            "abT": np.ascontiguousarray(ab[rows].T),
            "WlinT8": WlinT8,
            "WresT": WresT,
            "WoT": np.ascontiguousarray(Wo[:, 128 * c:128 * (c + 1)].T),
            "ident": ident,
            "ones": np.ones((128, 2), f32),
            "bqb": np.ascontiguousarray(np.broadcast_to(bq[colperm], (128, H))),
            "bkb": bkb,
            "blinb": blinb,
            "bres2b": bres2b,
            "bob": bob,
        })
    return in_maps


def combine_outputs(results):
    full = np.zeros((L, H), np.float32)
    for c in range(N_CORES):
        o = results[c]["out"]
        perm = _perm16(c)
        for j in range(16):
            full[128 * perm[j]:128 * (perm[j] + 1)] += o[128 * j:128 * (j + 1)]
    return full


def kernel(**inputs):
    nc = get_nc()
    in_maps = build_in_maps(inputs)
    res = bass_utils.run_bass_kernel_spmd(nc, in_maps,
                                          core_ids=list(range(N_CORES)))
    return combine_outputs(res.results)


# revision 6
# speedup vs baseline: 1.2837x; 1.2837x over previous
"""Trainium2 Bass kernel for nn_BiasedMultiHeadAtten (8-core SPMD, tensor
parallel over heads).

The torch module's transpose(0,1)+reshape "scramble" means head n of the
attention only reads rows [64n,64n+64) u [1024+64n,1024+64n+64) of q/k, and
the per-head attention factors into four 1024x1024 score blocks with
contraction 64.  Sharding 2 heads per core therefore also shards the q/k
projections 8-way (256 of 2048 rows each).

Per core c (heads 2c, 2c+1):
  - project q,k for its 256 rows (contraction 4096, bf16 PE, fp32 psum)
  - scrambled attention: S^T = Y^T X per (a0,b0,b1-block), exp on ACT
    (no max subtraction: |scores| <= ~11), AV via PE with a ones-column
    appended to V^T producing the softmax denominators for free
  - out-proj partial: o_cols @ Wo[:,cols]^T  (full 2048 rows)
  - gated-residual branch for its 256 rows, added into the two row-tiles
    that the host-side feature permutation pins at tile 0/1
Host sums the 8 partial outputs with per-core row un-permutation.
"""

import numpy as np
import ml_dtypes

import concourse.bacc as bacc
import concourse.mybir as mybir
import concourse.tile as tile
from concourse import bass_utils

N_CORES = 8
L, H, E, E2, HD = 2048, 1024, 4096, 2048, 64
F32 = mybir.dt.float32
BF16 = mybir.dt.bfloat16
AF = mybir.ActivationFunctionType

_NC_CACHE = {}


def _perm16(c):
    """Block permutation: device l-tile j holds global l-tile perm[j];
    perm[0] = c and perm[1] = 8 + c so the residual rows sit at tiles 0,1."""
    perm = list(range(16))

    def place(pos, val):
        i = perm.index(val)
        perm[pos], perm[i] = perm[i], perm[pos]

    place(0, c)
    place(1, 8 + c)
    return perm


def _emit(nc, tc, d, out):
    from contextlib import ExitStack

    with ExitStack() as ctx:
        pers = ctx.enter_context(tc.tile_pool(name="pers", bufs=1))

        X = [pers.tile([128, H], BF16, tag=f"X{a}", name=f"X{a}")
             for a in range(2)]
        Y = [pers.tile([128, H], BF16, tag=f"Y{b}", name=f"Y{b}")
             for b in range(2)]
        VT = [[pers.tile([128, 130], BF16, tag=f"VT{b}_{j}", name=f"VT{b}_{j}")
               for j in range(8)] for b in range(2)]
        ocolsT = pers.tile([128, 1024, 2], BF16, tag="ocolsT", name="ocolsT")
        bias = {}

        # ---- Phase A: q/k projections for this core's 256 rows ----------
        # q-pass then k-pass (4 psum banks each) so banks stay available
        # and the PE stream is dense while weights stream in.
        with tc.tile_pool(name="phA", bufs=3) as pA, \
             tc.tile_pool(name="phN", bufs=1) as pN, \
             tc.tile_pool(name="psA", bufs=1, space="PSUM") as psA:
            nodeT_sb = []
            for e in range(32):
                t = pN.tile([128, 256], BF16, tag=f"node{e}", name=f"node{e}")
                nc.sync.dma_start(t[:], d["nodeT"][128 * e:128 * (e + 1), :])
                nodeT_sb.append(t)
            for bn in ("bqb", "bkb", "blinb", "bres2b", "bob"):
                bias[bn] = pers.tile([128, H], F32, tag=bn, name=bn)
                nc.sync.dma_start(bias[bn][:], d[bn][:])
            for w, ps_tag, xy, bb in (("WqT", "q", X, "bqb"),
                                      ("WkT", "k", Y, "bkb")):
                ps = [[psA.tile([128, 512], F32, tag=f"{ps_tag}{lb}{ch}",
                                name=f"{ps_tag}{lb}{ch}")
                       for ch in range(2)] for lb in range(2)]
                for e in range(32):
                    wt = pA.tile([128, H], BF16, tag=f"w{ps_tag}",
                                 name=f"w{ps_tag}")
                    nc.sync.dma_start(wt[:], d[w][128 * e:128 * (e + 1), :])
                    st, sp = (e == 0), (e == 31)
                    for lb in range(2):
                        lhs = nodeT_sb[e][:, 128 * lb:128 * (lb + 1)]
                        for ch in range(2):
                            nc.tensor.matmul(ps[lb][ch][:], lhs,
                                             wt[:, 512 * ch:512 * (ch + 1)],
                                             start=st, stop=sp)
                for lb in range(2):
                    for ch in range(2):
                        sl = slice(512 * ch, 512 * (ch + 1))
                        nc.vector.tensor_add(xy[lb][:, sl], ps[lb][ch][:],
                                             bias[bb][:, sl])

        ident = pers.tile([128, 128], BF16, tag="ident", name="ident")
        nc.sync.dma_start(ident[:], d["ident"][:])
        ones_sb = pers.tile([128, 2], BF16, tag="ones", name="ones")
        nc.sync.dma_start(ones_sb[:], d["ones"][:])
        wo_sb = pers.tile([128, H], BF16, tag="wo", name="wo")
        nc.sync.dma_start(wo_sb[:], d["WoT"][:])

        # ---- Phase R: gated residual branch (fills PE gaps) -------------
        resg = [pers.tile([128, H], F32, tag=f"resg{lb}", name=f"resg{lb}")
                for lb in range(2)]
        with tc.tile_pool(name="pR", bufs=2) as pR, \
             tc.tile_pool(name="pAB", bufs=1) as pAB, \
             tc.tile_pool(name="psR", bufs=2, space="PSUM") as psR, \
             tc.tile_pool(name="psR2", bufs=1, space="PSUM") as psR2:
            abt_sb = []
            for t in range(16):
                a = pAB.tile([128, 256], BF16, tag=f"ab{t}", name=f"ab{t}")
                nc.sync.dma_start(a[:], d["abT"][128 * t:128 * (t + 1), :])
                abt_sb.append(a)
            res1T = [pers.tile([128, 256], BF16, tag=f"r1_{hb}",
                               name=f"r1_{hb}") for hb in range(8)]
            # res1^T[h, l'] accumulation, h-block outer / e2 inner
            for hb in range(8):
                wl = pR.tile([128, 16, 128], BF16, tag="wl", name="wl")
                nc.sync.dma_start(wl[:], d["WlinT8"][hb])
                rp = psR.tile([128, 256], F32, tag="rp", name="rp")
                for t in range(16):
                    nc.tensor.matmul(rp[:], wl[:, t, :], abt_sb[t][:],
                                     start=(t == 0), stop=(t == 15))
                nc.vector.tensor_copy(res1T[hb][:], rp[:])
            # res2 = res1' @ Wres^T (+ blin folded into bres2b)
            rp2 = [[psR2.tile([128, 512], F32, tag=f"rp2{lb}{ch}",
                              name=f"rp2{lb}{ch}")
                    for ch in range(2)] for lb in range(2)]
            for hb in range(8):
                wr = pR.tile([128, H], BF16, tag="wr", name="wr")
                nc.sync.dma_start(wr[:], d["WresT"][128 * hb:128 * (hb + 1), :])
                for lb in range(2):
                    for ch in range(2):
                        nc.tensor.matmul(rp2[lb][ch][:],
                                         res1T[hb][:, 128 * lb:128 * (lb + 1)],
                                         wr[:, 512 * ch:512 * (ch + 1)],
                                         start=(hb == 0), stop=(hb == 7))
            g_sb = []
            for lb in range(2):
                tt = pR.tile([128, H], F32, tag="tt", name="tt")
                for ch in range(2):
                    sl = slice(512 * ch, 512 * (ch + 1))
                    nc.vector.tensor_add(tt[:, sl], rp2[lb][ch][:],
                                         bias["bres2b"][:, sl])
                g = pers.tile([128, H], F32, tag=f"g{lb}", name=f"g{lb}")
                nc.scalar.activation(g[:], tt[:], AF.Sigmoid)
                g_sb.append(g)
            # res = (res1 + blin) * sigmoid(res2), transposed to (l', h)
            for lb in range(2):
                for hb in range(8):
                    tp = psR.tile([128, 128], BF16, tag="tp2", name="tp2")
                    nc.tensor.transpose(tp[:],
                                        res1T[hb][:, 128 * lb:128 * (lb + 1)],
                                        ident[:])
                    sl = slice(128 * hb, 128 * (hb + 1))
                    nc.vector.tensor_add(resg[lb][:, sl], tp[:],
                                         bias["blinb"][:, sl])
                    nc.vector.tensor_mul(resg[lb][:, sl], resg[lb][:, sl],
                                         g_sb[lb][:, sl])
                nc.vector.tensor_add(resg[lb][:], resg[lb][:], bias["bob"][:])

        # ---- Phase B: V^T tiles (PE transpose), ones col for denom ------
        with tc.tile_pool(name="psT", bufs=2, space="PSUM") as psT:
            for b0 in range(2):
                for j in range(8):
                    pt = psT.tile([128, 128], BF16, tag="tp", name="tp")
                    nc.tensor.transpose(pt[:], Y[b0][:, 128 * j:128 * (j + 1)],
                                        ident[:])
                    vt = VT[b0][j]
                    nc.vector.tensor_copy(vt[:, 0:64], pt[:, 0:64])
                    nc.vector.tensor_copy(vt[:, 64:65], ones_sb[:, 0:1])
                    nc.vector.tensor_copy(vt[:, 65:129], pt[:, 64:128])
                    nc.vector.tensor_copy(vt[:, 129:130], ones_sb[:, 1:2])

        # ---- Phase C: scrambled attention ------------------------------
        with tc.tile_pool(name="pP", bufs=2) as pP, \
             tc.tile_pool(name="pM", bufs=2) as pM, \
             tc.tile_pool(name="psS", bufs=1, space="PSUM") as psS, \
             tc.tile_pool(name="psO", bufs=1, space="PSUM") as psO:
            for a0 in range(2):
                O_ps = [[psO.tile([65, 512], F32, tag=f"O{h}{ch}",
                                  name=f"O{h}{ch}")
                         for ch in range(2)] for h in range(2)]
                for b0 in range(2):
                    for j in range(8):
                        bt = 8 * b0 + j
                        s_ps = [psS.tile([128, 1024], F32, tag=f"s{h}",
                                         name=f"s{h}") for h in range(2)]
                        for h in range(2):
                            hp = slice(64 * h, 64 * (h + 1))
                            for ch in range(2):
                                nc.tensor.matmul(
                                    s_ps[h][:, 512 * ch:512 * (ch + 1)],
                                    Y[b0][hp, 128 * j:128 * (j + 1)],
                                    X[a0][hp, 512 * ch:512 * (ch + 1)],
                                    start=True, stop=True)
                        p_sb = [pP.tile([128, 1024], BF16, tag=f"p{h}",
                                        name=f"p{h}") for h in range(2)]
                        for h in range(2):
                            nc.scalar.activation(p_sb[h][:], s_ps[h][:],
                                                 AF.Exp, scale=0.125)
                        for h in range(2):
                            for ch in range(2):
                                nc.tensor.matmul(
                                    O_ps[h][ch][:],
                                    VT[b0][j][:, 65 * h:65 * (h + 1)],
                                    p_sb[h][:, 512 * ch:512 * (ch + 1)],
                                    start=(bt == 0), stop=(bt == 15))
                # normalize by the ones-row denominators, scatter into ocolsT
                for h in range(2):
                    r_sb = pM.tile([1, 1024], F32, tag="r", name="r")
                    for ch in range(2):
                        nc.vector.tensor_copy(r_sb[:, 512 * ch:512 * (ch + 1)],
                                              O_ps[h][ch][64:65, :])
                    rcp = pM.tile([1, 1024], F32, tag="rcp", name="rcp")
                    nc.vector.reciprocal_approx_fast(rcp[:], r_sb[:])
                    rcpb = pM.tile([64, 1024], F32, tag="rcpb", name="rcpb")
                    nc.gpsimd.partition_broadcast(rcpb[:], rcp[:])
                    for ch in range(2):
                        nc.vector.tensor_mul(
                            ocolsT[64 * h:64 * (h + 1),
                                   512 * ch:512 * (ch + 1), a0],
                            O_ps[h][ch][0:64, :],
                            rcpb[:, 512 * ch:512 * (ch + 1)])

        # ---- Phase O: out-projection partial + residual/bias add --------
        with tc.tile_pool(name="pO", bufs=2) as pO, \
             tc.tile_pool(name="psF", bufs=2, space="PSUM") as psF:
            oc_flat = ocolsT[:].rearrange("p a b -> p (a b)")
            for j in range(16):
                op = psF.tile([128, 1024], F32, tag="op", name="op")
                for ch in range(2):
                    nc.tensor.matmul(op[:, 512 * ch:512 * (ch + 1)],
                                     oc_flat[:, 128 * j:128 * (j + 1)],
                                     wo_sb[:, 512 * ch:512 * (ch + 1)],
                                     start=True, stop=True)
                ob = pO.tile([128, H], F32, tag="ob", name="ob")
                if j < 2:
                    nc.vector.tensor_add(ob[:], op[:], resg[j][:])
                else:
                    nc.vector.tensor_copy(ob[:], op[:])
                nc.sync.dma_start(out[128 * j:128 * (j + 1), :], ob[:])


def _build_nc():
    nc = bacc.Bacc("TRN2", target_bir_lowering=False, debug=False,
                   num_devices=N_CORES)
    d = {}

    def din(name, shape, dt=BF16):
        d[name] = nc.dram_tensor(name, shape, dt, kind="ExternalInput").ap()

    din("nodeT", (E, 256))
    din("WqT", (E, H))
    din("WkT", (E, H))
    din("abT", (E2, 256))
    din("WlinT8", (8, 128, 16, 128))
    din("WresT", (H, H))
    din("WoT", (128, H))
    din("ident", (128, 128))
    din("ones", (128, 2))
    for bn in ("bqb", "bkb", "blinb", "bres2b", "bob"):
        din(bn, (128, H), F32)
    out = nc.dram_tensor("out", (L, H), F32, kind="ExternalOutput").ap()
    with tile.TileContext(nc) as tc:
        _emit(nc, tc, d, out)
    nc.compile()
    return nc


def get_nc():
    if "nc" not in _NC_CACHE:
        _NC_CACHE["nc"] = _build_nc()
    return _NC_CACHE["nc"]


def build_in_maps(inputs):
    f32 = np.float32
    bf16 = ml_dtypes.bfloat16
    ne = np.asarray(inputs["node_embedding"], f32)
    ab = np.asarray(inputs["atten_bias"], f32)
    Wq = np.asarray(inputs["Wq"], f32)
    Wk = np.asarray(inputs["Wk"], f32)
    Wlin = np.asarray(inputs["Wlin"], f32)
    Wres = np.asarray(inputs["Wres"], f32)
    Wo = np.asarray(inputs["Wo"], f32)
    bq = np.asarray(inputs["bq"], f32)
    bk = np.asarray(inputs["bk"], f32)
    blin = np.asarray(inputs["blin"], f32)
    bres = np.asarray(inputs["bres"], f32)
    bo = np.asarray(inputs["bo"], f32)

    WkT = np.ascontiguousarray(Wk.T).astype(bf16)
    WlinT8 = np.ascontiguousarray(
        Wlin.T.reshape(16, 128, 8, 128).transpose(2, 1, 0, 3)).astype(bf16)
    WresT = np.ascontiguousarray(Wres.T).astype(bf16)
    ident = np.eye(128, dtype=f32).astype(bf16)
    ones = np.ones((128, 2), f32).astype(bf16)
    bres2 = Wres @ blin + bres
    bkb = np.ascontiguousarray(np.broadcast_to(bk, (128, H)))
    blinb = np.ascontiguousarray(np.broadcast_to(blin, (128, H)))
    bres2b = np.ascontiguousarray(np.broadcast_to(bres2, (128, H)))
    bob = np.ascontiguousarray(np.broadcast_to(bo, (128, H)))

    in_maps = []
    for c in range(N_CORES):
        rows = np.r_[128 * c:128 * (c + 1),
                     1024 + 128 * c:1024 + 128 * (c + 1)]
        colperm = np.concatenate([np.arange(64) + 64 * p for p in _perm16(c)])
        in_maps.append({
            "nodeT": np.ascontiguousarray(ne[rows].T).astype(bf16),
            "WqT": np.ascontiguousarray(Wq.T[:, colperm]).astype(bf16),
            "WkT": WkT,
            "abT": np.ascontiguousarray(ab[rows].T).astype(bf16),
            "WlinT8": WlinT8,
            "WresT": WresT,
            "WoT": np.ascontiguousarray(
                Wo[:, 128 * c:128 * (c + 1)].T).astype(bf16),
            "ident": ident,
            "ones": ones,
            "bqb": np.ascontiguousarray(np.broadcast_to(bq[colperm], (128, H))),
            "bkb": bkb,
            "blinb": blinb,
            "bres2b": bres2b,
            "bob": bob,
        })
    return in_maps


def combine_outputs(results):
    full = np.zeros((L, H), np.float32)
    for c in range(N_CORES):
        o = results[c]["out"]
        perm = _perm16(c)
        for j in range(16):
            full[128 * perm[j]:128 * (perm[j] + 1)] += o[128 * j:128 * (j + 1)]
    return full


def kernel(**inputs):
    nc = get_nc()
    in_maps = build_in_maps(inputs)
    res = bass_utils.run_bass_kernel_spmd(nc, in_maps,
                                          core_ids=list(range(N_CORES)))
    return combine_outputs(res.results)
